# revision 14
# baseline (speedup 1.0000x reference)
"""GATv2 (2-layer, graph-norm) Trainium2 Bass kernel, v2.

B=8 samples of N=1024 nodes; data-parallel one sample per NeuronCore (8
cores). Full inputs in, full output out.

Math notes (same reductions as v1):
- GATv2 score e[i,j] = sl[i] + sr[j]; sl cancels in the row softmax, so
  att[i,:] = adj[i,:]*exp(sr) / (adj[i,:] @ exp(sr)). Left branch never
  needed. exp args are small (|t| < 13), no max subtraction.
- Augmented value matrix V = [w*R | w] makes one adj@V matmul produce
  numerator and denominator together.

v2 layout insight: load adj PACKED [128p, 8k, 1024j] (row 8p+k on
partition p; one DMA, 32KB contiguous descriptors -- the v1 row-major
load had 4KB descriptors and ran at ~60 GB/s aggregate). PE-transpose
slice k of the packed tile directly: out[j, p] = adj[8p+k, j], i.e.
adjT columns i = 8p+k (i-strided). Using those as matmul lhsT makes
output tile k hold rows i = 8p+k = pseudo-nodes 8m+g with g=k -- which
is exactly channel-group k of ALL nodes in node-major order. So layer-1
outputs land node-major with partition = n%128 and NO DRAM staging
round-trip for the output reshuffle (v1 staged 1 MB through DRAM).

Precision: adjacency and V matrices in bf16 (adj 0/1 exact; V errors
~0.4% average out over ~512-neighbor sums); score path (R -> t -> w =
exp(t)) kept fp32 since exp amplifies absolute t errors.
"""
import numpy as np
from contextlib import ExitStack

import concourse.bass as bass
import concourse.tile as tile
import concourse.mybir as mybir
from concourse.masks import make_identity

F32 = mybir.dt.float32
BF16 = mybir.dt.bfloat16
INT32 = mybir.dt.int32
import os as _os
MMDT = F32 if _os.environ.get("GAT_MMDT") == "f32" else BF16
AF = mybir.ActivationFunctionType
OP = mybir.AluOpType

N = 1024
NF = 128
NH1 = 128
NH2 = 256
NT = 8
EPS = 1e-5
SLOPE = 0.2

INPUT_KEYS = [
    ("x", (N, NF), F32),
    ("adj", (N, N), INT32),
    ("W_r1", (NF, NH1), F32),
    ("a1", (16, 1), F32),
    ("W_r2", (NH1, NH2), F32),
    ("a2", (NH2, 1), F32),
    ("gn1_scale", (NF,), F32),
    ("gn1_shift", (NF,), F32),
    ("gn1_alpha", (NF,), F32),
    ("gn2_scale", (NH1,), F32),
    ("gn2_shift", (NH1,), F32),
    ("gn2_alpha", (NH1,), F32),
]


def gat_body(ctx: ExitStack, tc: tile.TileContext, io: dict):
    nc = tc.nc
    const = ctx.enter_context(tc.tile_pool(name="const", bufs=1))
    big = ctx.enter_context(tc.tile_pool(name="big", bufs=1))
    work = ctx.enter_context(tc.tile_pool(name="work", bufs=3))
    small = ctx.enter_context(tc.tile_pool(name="small", bufs=4))
    psA = ctx.enter_context(tc.tile_pool(name="psA", bufs=2, space="PSUM"))
    psR = ctx.enter_context(tc.tile_pool(name="psR", bufs=2, space="PSUM"))
    psH = ctx.enter_context(tc.tile_pool(name="psH", bufs=4, space="PSUM"))
    dram = ctx.enter_context(tc.tile_pool(name="dram", bufs=1, space="DRAM"))

    # ---------------- input DMAs, longest pole first ----------------
    # x flat first (x-branch unblocks early); adj right behind on same ring.
    xg = big.tile([128, N], F32)
    nc.sync.dma_start(out=xg,
                      in_=io["x"].rearrange("(p k) c -> p (k c)", p=128))
    # adj packed: adjpk[p, k, j] = adj[8p+k, j]; one DMA, 32KB descriptors.
    adjpk = big.tile([128, NT, N], INT32)
    nc.sync.dma_start(out=adjpk,
                      in_=io["adj"].rearrange("(p k) j -> p k j", p=128))

    gn = {}
    for k in ("gn1_scale", "gn1_shift", "gn1_alpha",
              "gn2_scale", "gn2_shift", "gn2_alpha"):
        t = const.tile([128, 1], F32, tag=k)
        nc.scalar.dma_start(out=t, in_=io[k])
        gn[k] = t

    Wr1 = const.tile([128, NH1], F32)
    nc.scalar.dma_start(out=Wr1, in_=io["W_r1"])
    Wr2 = const.tile([128, NH2], F32)
    nc.scalar.dma_start(out=Wr2, in_=io["W_r2"])

    # a1[d] tiled over (q, d): [128, 1024]
    a1rep = const.tile([128, N], F32)
    nc.scalar.dma_start(out=a1rep.rearrange("p (q d) -> p q d", d=16),
                        in_=bass.AP(tensor=io["a1"].tensor,
                                    offset=io["a1"].offset,
                                    ap=[[0, 128], [0, 64], [1, 16]]))
    # a2 on every partition: [128, 256]
    a2rep = const.tile([128, NH2], F32)
    nc.scalar.dma_start(out=a2rep,
                        in_=bass.AP(tensor=io["a2"].tensor,
                                    offset=io["a2"].offset,
                                    ap=[[0, 128], [1, NH2]]))

    # ---------------- constants ----------------
    identb = const.tile([128, 128], MMDT)
    make_identity(nc, identb)
    identf = const.tile([128, 128], F32)
    make_identity(nc, identf)
    eps_t = const.tile([128, 1], F32)
    nc.vector.memset(eps_t, EPS)
    neg1 = const.tile([128, 1], F32)
    nc.vector.memset(neg1, -1.0)
    # sel16[p, q] = (p // 8 == q), bf16 -- group-of-8-partitions selector
    sel16 = const.tile([128, 16], MMDT)
    nc.gpsimd.memset(sel16, 0.0)
    nc.gpsimd.affine_select(out=sel16, in_=sel16, compare_op=OP.is_ge,
                            fill=1.0, base=-1, pattern=[[8, 16]],
                            channel_multiplier=-1)
    nc.gpsimd.affine_select(out=sel16, in_=sel16, compare_op=OP.is_ge,
                            fill=0.0, base=7, pattern=[[8, 16]],
                            channel_multiplier=-1)

    # ---------------- adj cast + transpose ----------------
    # adjf[k][p, j] = float(adj[8p+k, j]), bf16. int32->f32 on DVE (the
    # one proven convert path), f32->bf16 on ACT.
    adjf = []
    for k in range(NT):
        t = big.tile([128, N], MMDT, tag=f"adjf{k}", name=f"adjf{k}")
        nc.vector.tensor_copy(t, adjpk[:, k, :])
        adjf.append(t)

    # adjT[jl, jt, k, p] = adj[8p+k, 128jt+jl]  (lhsT tiles, bf16)
    adjT = big.tile([128, NT, NT, 128], MMDT)
    for k in range(NT):
        for jg in range(0, NT, 4):
            pst = psA.tile([128, 4, 128], MMDT, tag="pst", name=f"pst{k}_{jg}")
            for j in range(4):
                nc.tensor.transpose(pst[:, j, :],
                                    adjf[k][:, 128 * (jg + j):128 * (jg + j + 1)],
                                    identb)
            dst = adjT[:, jg:jg + 4, k, :]
            if (k + jg) % 2 == 0:
                nc.vector.tensor_copy(dst, pst)
            else:
                nc.scalar.copy(dst, pst)

    # ---------------- layer 1: graph_norm ----------------
    stats = small.tile([128, 2, 6], F32)
    nc.vector.bn_stats(stats[:, 0, :], xg[:, 0:512])
    nc.vector.bn_stats(stats[:, 1, :], xg[:, 512:1024])
    mv = small.tile([128, 2], F32)
    nc.vector.bn_aggr(mv, stats)
    lnv = small.tile([128, 1], F32)
    nc.scalar.activation(lnv, mv[:, 1:2], AF.Ln, bias=eps_t)
    rstd = small.tile([128, 1], F32)
    nc.scalar.activation(rstd, lnv, AF.Exp, scale=-0.5)
    S1 = small.tile([128, 1], F32)
    nc.vector.tensor_mul(S1, rstd, gn["gn1_scale"])
    t0 = small.tile([128, 1], F32)
    nc.vector.tensor_mul(t0, mv[:, 0:1], S1)
    t1 = small.tile([128, 1], F32)
    nc.vector.tensor_mul(t1, t0, gn["gn1_alpha"])
    B1 = small.tile([128, 1], F32)
    nc.vector.tensor_sub(B1, gn["gn1_shift"], t1)
    h1g = big.tile([128, N], F32)
    nc.vector.tensor_scalar(out=h1g, in0=xg, scalar1=S1, scalar2=B1,
                            op0=OP.mult, op1=OP.add)

    # h1T[c, k, p] = h1[8p+k, c]
    h1T = big.tile([128, NT, 128], F32)
    for k in range(NT):
        pst = psA.tile([128, 128], F32, tag="pst")
        nc.tensor.transpose(pst, h1g[:, 128 * k:128 * (k + 1)], identf)
        nc.scalar.copy(h1T[:, k, :], pst)

    # R_all[p, k, :] = leaky(h1 @ W_r1)[8p+k, :]   (fp32 score path)
    R_all = big.tile([128, NT, NH1], F32)
    for k in range(NT):
        psr = psR.tile([128, NH1], F32, tag="psr")
        nc.tensor.matmul(psr, h1T[:, k, :], Wr1, start=True, stop=True)
        rcp = work.tile([128, NH1], F32, tag="rcp1")
        nc.scalar.copy(rcp, psr)
        nc.vector.scalar_tensor_tensor(
            out=R_all[:, k, :], in0=rcp, scalar=SLOPE, in1=rcp,
            op0=OP.mult, op1=OP.max)

    # t[p, (k g)] = sum_d R[p, k, 16g+d] * a1[d];  w = exp(t)
    tmul = big.tile([128, N], F32)
    nc.vector.tensor_mul(tmul, R_all.rearrange("p k c -> p (k c)"), a1rep)
    t_all = big.tile([128, 64], F32)
    nc.vector.tensor_reduce(
        out=t_all, in_=tmul.rearrange("p (q d) -> p q d", d=16),
        axis=mybir.AxisListType.X, op=OP.add)
    w_all = big.tile([128, 64], F32)
    nc.scalar.activation(w_all, t_all, AF.Exp)

    # R17[p, k, 17g+(0:16)] = w*R, R17[p, k, 17g+16] = w   (bf16 values)
    R17 = big.tile([128, NT, 136], MMDT)
    v17 = R17.rearrange("p u (g x) -> p u g x", x=17)
    w3 = w_all.rearrange("p (u g) -> p u g", g=8)
    nc.vector.tensor_mul(v17[:, :, :, 0:16],
                         R_all.rearrange("p u (g d) -> p u g d", d=16),
                         w3.to_broadcast([128, 8, 8, 16]))
    nc.vector.tensor_copy(v17[:, :, :, 16], w3)

    # V1[j'-tile kt] from R17 via DRAM staging (bf16, halves v1's bytes):
    # stage addr A(h,kt,a,b,g,dd) = 17408h + 2176kt + 1088a + 136b + 17g + dd
    vstage = dram.tile([139264], MMDT)
    nc.sync.dma_start(
        out=bass.AP(tensor=vstage.tensor, offset=vstage.offset,
                    ap=[[17408, 8], [1088, 16], [1, 1088]]),
        in_=R17.rearrange("p u c -> p (u c)"))
    V1 = big.tile([128, NT, 136], MMDT)
    for kt in range(NT):
        nc.sync.dma_start(
            out=V1[:, kt, :],
            in_=bass.AP(tensor=vstage.tensor,
                        offset=vstage.offset + 2176 * kt,
                        ap=[[17, 128], [17408, 8], [1, 17]]))

    # ---------------- layer 1: hp = adj @ V1, node-major epilogue ------
    # out tile k rows = nodes i = 8p+k (pseudo 8m+g with g=k), so the
    # normalized/elu'd result IS node-major: out1f[m, h, 16k+d].
    # out1f holds elu+1 (the -1 is folded into B2 / stats downstream).
    out1f = big.tile([128, NT, 128], F32)
    o3 = out1f.rearrange("p h (g d) -> p h g d", d=16)
    for kg in range(0, NT, 4):
        pss = {}
        for k in range(kg, kg + 4):
            pss[k] = psH.tile([128, 136], F32, tag="ps", name=f"hp1_{k}")
        for jt in range(NT):
            for k in range(kg, kg + 4):
                nc.tensor.matmul(pss[k], adjT[:, jt, k, :], V1[:, jt, :],
                                 start=(jt == 0), stop=(jt == NT - 1))
        for k in range(kg, kg + 4):
            ps = pss[k]
            p3 = ps.rearrange("p (h x) -> p h x", x=17)
            rec = work.tile([128, 8], F32, tag="rec1")
            nc.vector.reciprocal(rec, p3[:, :, 16])
            hpn = work.tile([128, 128], F32, tag="hpn")
            nc.vector.tensor_mul(hpn.rearrange("p (h d) -> p h d", d=16),
                                 p3[:, :, 0:16], rec.to_broadcast([128, 8, 16]))
            # elu+1 = relu(x) + exp(min(x, 0)), written strided node-major
            mn = work.tile([128, 128], F32, tag="mn1")
            nc.vector.tensor_scalar_min(out=mn, in0=hpn, scalar1=0.0)
            ex = work.tile([128, 128], F32, tag="ex1")
            nc.scalar.activation(ex, mn, AF.Exp)
            nc.vector.scalar_tensor_tensor(
                out=o3[:, :, k, :],
                in0=hpn.rearrange("p (h d) -> p h d", d=16),
                scalar=0.0, in1=ex.rearrange("p (h d) -> p h d", d=16),
                op0=OP.max, op1=OP.add)

    # bf16 copy + squares (stats / V2-source matmul operands)
    out1b = big.tile([128, N], MMDT)
    nc.vector.tensor_copy(out1b, out1f.rearrange("p h c -> p (h c)"))
    sq = big.tile([128, N], MMDT)
    nc.vector.tensor_mul(sq, out1b, out1b)

    # group sums: psS[q, (h c)] = sum_{p: p//8==q} out1b[p, (h c)]
    s2sum = small.tile([128, 1], F32, tag="s2sum")
    q2sum = small.tile([128, 1], F32, tag="q2sum")
    if _os.environ.get("GAT_STUB_STATS"):
        # crash-bisect stub: fake stats (numerics wrong, mean'=1 var=1)
        nc.vector.memset(s2sum, 1024.0)
        nc.vector.memset(q2sum, 2048.0)
        # keep readers of out1b/sq alive so schedule shape stays similar
        junk = small.tile([128, 1], F32, tag="junk")
        nc.vector.tensor_reduce(out=junk, in_=sq[:, 0:8],
                                axis=mybir.AxisListType.X, op=OP.add)
    else:
        s16 = small.tile([16, 16], F32, tag="s16")
        for srcb, base in ((out1b, 0), (sq, 8)):
            for half in range(2):
                psS = psH.tile([16, 512], F32, tag="ps", name=f"psS{base}_{half}")
                nc.tensor.matmul(psS, sel16, srcb[:, 512 * half:512 * (half + 1)],
                                 start=True, stop=True)
                nc.vector.tensor_reduce(
                    out=s16[:, base + 4 * half:base + 4 * half + 4],
                    in_=psS.rearrange("p (h c) -> p h c", c=128),
                    axis=mybir.AxisListType.X, op=OP.add)
        psst = psA.tile([16, 16], F32, tag="pst")
        nc.tensor.transpose(psst, s16, identf[0:16, 0:16])
        st = small.tile([16, 16], F32, tag="st")
        nc.scalar.copy(st, psst)
        nc.sync.dma_start(out=s2sum, in_=st[0:8, :])
        nc.sync.dma_start(out=q2sum, in_=st[8:16, :])

    # ---------------- layer 2: graph_norm scales ----------------
    # values stored are o1' = elu+1; var is shift-invariant, mean' = mean+1,
    # and B2 absorbs the -1: h2 = S2*o1' + (shift - S2*(1 + alpha*(mean'-1)))
    inv = 1.0 / 1024.0
    mean2 = small.tile([128, 1], F32, tag="mean2")
    nc.vector.tensor_scalar_mul(mean2, s2sum, inv)
    ex2m = small.tile([128, 1], F32, tag="ex2m")
    nc.vector.tensor_scalar_mul(ex2m, q2sum, inv)
    msq = small.tile([128, 1], F32, tag="msq")
    nc.vector.tensor_mul(msq, mean2, mean2)
    var2 = small.tile([128, 1], F32, tag="var2")
    nc.vector.tensor_sub(var2, ex2m, msq)
    lnv2 = small.tile([128, 1], F32, tag="lnv2")
    nc.scalar.activation(lnv2, var2, AF.Ln, bias=eps_t)
    rstd2 = small.tile([128, 1], F32, tag="rstd2")
    nc.scalar.activation(rstd2, lnv2, AF.Exp, scale=-0.5)
    S2 = small.tile([128, 1], F32, tag="S2")
    nc.vector.tensor_mul(S2, rstd2, gn["gn2_scale"])
    m1 = small.tile([128, 1], F32, tag="m1")
    nc.vector.tensor_scalar_add(m1, mean2, -1.0)
    u0 = small.tile([128, 1], F32, tag="u0")
    nc.vector.tensor_mul(u0, m1, gn["gn2_alpha"])
    u1 = small.tile([128, 1], F32, tag="u1")
    nc.vector.tensor_scalar_add(u1, u0, 1.0)
    u2 = small.tile([128, 1], F32, tag="u2")
    nc.vector.tensor_mul(u2, u1, S2)
    B2 = small.tile([128, 1], F32, tag="B2")
    nc.vector.tensor_sub(B2, gn["gn2_shift"], u2)

    # ---------------- layer 2: R2, w2, V2 ----------------
    h2T = big.tile([128, NT, 128], F32)
    R2f = big.tile([128, NT, NH2], F32)
    t2 = big.tile([128, NT], F32)
    sc2 = big.tile([128, NH2], F32)
    for ht in range(NT):
        S2c = work.tile([128, 1], F32, tag="s2c")
        nc.scalar.dma_start(out=S2c,
                            in_=S2[16 * ht:16 * ht + 16, 0].to_broadcast([16, 8]))
        B2c = work.tile([128, 1], F32, tag="b2c")
        nc.scalar.dma_start(out=B2c,
                            in_=B2[16 * ht:16 * ht + 16, 0].to_broadcast([16, 8]))
        h2t = work.tile([128, 128], F32, tag="h2t")
        nc.vector.tensor_scalar(out=h2t, in0=out1f[:, ht, :], scalar1=S2c,
                                scalar2=B2c, op0=OP.mult, op1=OP.add)
        pst = psA.tile([128, 128], F32, tag="pst")
        nc.tensor.transpose(pst, h2t, identf)
        nc.scalar.copy(h2T[:, ht, :], pst)
        psr = psR.tile([128, NH2], F32, tag="psr")
        nc.tensor.matmul(psr, h2T[:, ht, :], Wr2, start=True, stop=True)
        rcp2 = work.tile([128, NH2], F32, tag="rcp2")
        nc.scalar.copy(rcp2, psr)
        nc.vector.scalar_tensor_tensor(
            out=R2f[:, ht, :], in0=rcp2, scalar=SLOPE, in1=rcp2,
            op0=OP.mult, op1=OP.max)
        nc.vector.tensor_mul(sc2, R2f[:, ht, :], a2rep)
        nc.vector.tensor_reduce(out=t2[:, ht:ht + 1], in_=sc2,
                                axis=mybir.AxisListType.X, op=OP.add)
    w2 = big.tile([128, NT], F32)
    nc.scalar.activation(w2, t2, AF.Exp)

    V2 = big.tile([128, NT, NH2 + 1], MMDT)
    for kt in range(NT):
        nc.vector.tensor_scalar_mul(out=V2[:, kt, 0:NH2], in0=R2f[:, kt, :],
                                    scalar1=w2[:, kt:kt + 1])
    nc.vector.tensor_copy(V2[:, :, NH2], w2)

    # ---------------- layer 2: hp2 = adj @ V2, elu, y ----------------
    # out tile k rows = nodes i = 8p+k; y written with strided row DMA.
    yoff = io["y"].offset
    for kg in range(0, NT, 4):
        pss = {}
        for k in range(kg, kg + 4):
            pss[k] = psH.tile([128, NH2 + 1], F32, tag="ps", name=f"hp2_{k}")
        for jt in range(NT):
            for k in range(kg, kg + 4):
                nc.tensor.matmul(pss[k], adjT[:, jt, k, :], V2[:, jt, :],
                                 start=(jt == 0), stop=(jt == NT - 1))
        for k in range(kg, kg + 4):
            ps = pss[k]
            rec2 = work.tile([128, 1], F32, tag="rec2")
            nc.vector.reciprocal(rec2, ps[:, NH2:NH2 + 1])
            y0 = work.tile([128, NH2], F32, tag="y0")
            nc.vector.tensor_scalar_mul(out=y0, in0=ps[:, 0:NH2], scalar1=rec2)
            mn2 = work.tile([128, NH2], F32, tag="mn2")
            nc.vector.tensor_scalar_min(out=mn2, in0=y0, scalar1=0.0)
            ex2 = work.tile([128, NH2], F32, tag="ex2")
            nc.scalar.activation(ex2, mn2, AF.Exp)
            o2 = work.tile([128, NH2], F32, tag="o2")
            nc.vector.scalar_tensor_tensor(
                out=o2, in0=y0, scalar=0.0, in1=ex2, op0=OP.max, op1=OP.add)
            yo = work.tile([128, NH2], F32, tag="yo")
            nc.scalar.activation(yo, o2, AF.Identity, bias=neg1)
            nc.scalar.dma_start(
                out=bass.AP(tensor=io["y"].tensor, offset=yoff + NH2 * k,
                            ap=[[NH2 * 8, 128], [1, NH2]]),
                in_=yo)


def build_program():
    from concourse import bacc

    nc = bacc.Bacc("TRN2", target_bir_lowering=False, debug=False,
                   enable_asserts=True, num_devices=8)
    io = {}
    for name, shape, dt in INPUT_KEYS:
        io[name] = nc.dram_tensor(name, list(shape), dt, kind="ExternalInput").ap()
    io["y"] = nc.dram_tensor("y", [N, NH2], F32, kind="ExternalOutput").ap()
    with tile.TileContext(nc) as tc:
        with ExitStack() as ctx:
            gat_body(ctx, tc, io)
    nc.compile()
    return nc


def _run(inputs, **spmd_kwargs):
    from concourse.bass_utils import run_bass_kernel_spmd

    nc = build_program()
    B = 8
    in_maps = []
    for b in range(B):
        m = {}
        for name, shape, dt in INPUT_KEYS:
            v = np.asarray(inputs[name])
            if name in ("x", "adj"):
                v = v[b]
            m[name] = np.ascontiguousarray(v.reshape(shape),
                                           dtype=mybir.dt.np(dt))
        in_maps.append(m)
    res = run_bass_kernel_spmd(nc, in_maps, core_ids=list(range(B)),
                               **spmd_kwargs)
    out = np.stack([res.results[b]["y"] for b in range(B)], axis=0)
    return out.astype(np.float32), res


def kernel(**inputs) -> np.ndarray:
    return _run(inputs)[0]


# revision 16
# speedup vs baseline: 1.5729x; 1.5729x over previous
"""GATv2 (2-layer, graph-norm) Trainium2 Bass kernel, v2.

B=8 samples of N=1024 nodes; data-parallel one sample per NeuronCore (8
cores). Full inputs in, full output out.

Math notes (same reductions as v1):
- GATv2 score e[i,j] = sl[i] + sr[j]; sl cancels in the row softmax, so
  att[i,:] = adj[i,:]*exp(sr) / (adj[i,:] @ exp(sr)). Left branch never
  needed. exp args are small (|t| < 13), no max subtraction.
- Augmented value matrix V = [w*R | w] makes one adj@V matmul produce
  numerator and denominator together.

v2 layout insight: load adj PACKED [128p, 8k, 1024j] (row 8p+k on
partition p; one DMA, 32KB contiguous descriptors -- the v1 row-major
load had 4KB descriptors and ran at ~60 GB/s aggregate). PE-transpose
slice k of the packed tile directly: out[j, p] = adj[8p+k, j], i.e.
adjT columns i = 8p+k (i-strided). Using those as matmul lhsT makes
output tile k hold rows i = 8p+k = pseudo-nodes 8m+g with g=k -- which
is exactly channel-group k of ALL nodes in node-major order. So layer-1
outputs land node-major with partition = n%128 and NO DRAM staging
round-trip for the output reshuffle (v1 staged 1 MB through DRAM).

Precision: adjacency and V matrices in bf16 (adj 0/1 exact; V errors
~0.4% average out over ~512-neighbor sums); score path (R -> t -> w =
exp(t)) kept fp32 since exp amplifies absolute t errors.
"""
import numpy as np
from contextlib import ExitStack

import concourse.bass as bass
import concourse.tile as tile
import concourse.mybir as mybir
from concourse.masks import make_identity

F32 = mybir.dt.float32
BF16 = mybir.dt.bfloat16
INT32 = mybir.dt.int32
import os as _os
MMDT = F32 if _os.environ.get("GAT_MMDT") == "f32" else BF16
AF = mybir.ActivationFunctionType
OP = mybir.AluOpType

N = 1024
NF = 128
NH1 = 128
NH2 = 256
NT = 8
EPS = 1e-5
SLOPE = 0.2

INPUT_KEYS = [
    ("x", (N, NF), F32),
    ("adj", (N, N), INT32),
    ("W_r1", (NF, NH1), F32),
    ("a1", (16, 1), F32),
    ("W_r2", (NH1, NH2), F32),
    ("a2", (NH2, 1), F32),
    ("gn1_scale", (NF,), F32),
    ("gn1_shift", (NF,), F32),
    ("gn1_alpha", (NF,), F32),
    ("gn2_scale", (NH1,), F32),
    ("gn2_shift", (NH1,), F32),
    ("gn2_alpha", (NH1,), F32),
]


def gat_body(ctx: ExitStack, tc: tile.TileContext, io: dict):
    nc = tc.nc
    const = ctx.enter_context(tc.tile_pool(name="const", bufs=1))
    big = ctx.enter_context(tc.tile_pool(name="big", bufs=1))
    work = ctx.enter_context(tc.tile_pool(name="work", bufs=3))
    small = ctx.enter_context(tc.tile_pool(name="small", bufs=4))
    psA = ctx.enter_context(tc.tile_pool(name="psA", bufs=2, space="PSUM"))
    psR = ctx.enter_context(tc.tile_pool(name="psR", bufs=2, space="PSUM"))
    psH = ctx.enter_context(tc.tile_pool(name="psH", bufs=4, space="PSUM"))
    dram = ctx.enter_context(tc.tile_pool(name="dram", bufs=1, space="DRAM"))

    # ---------------- input DMAs, longest pole first ----------------
    # x flat first (x-branch unblocks early); adj right behind on same ring.
    xg = big.tile([128, N], F32)
    nc.sync.dma_start(out=xg,
                      in_=io["x"].rearrange("(p k) c -> p (k c)", p=128))
    # adj packed: adjpk[p, k, j] = adj[8p+k, j]; one DMA, 32KB descriptors.
    adjpk = big.tile([128, NT, N], INT32)
    nc.sync.dma_start(out=adjpk,
                      in_=io["adj"].rearrange("(p k) j -> p k j", p=128))

    gn = {}
    for k in ("gn1_scale", "gn1_shift", "gn1_alpha"):
        t = const.tile([128, 1], F32, tag=k)
        nc.sync.dma_start(out=t, in_=io[k])
        gn[k] = t
    Wr1 = const.tile([128, NH1], F32)
    nc.sync.dma_start(out=Wr1, in_=io["W_r1"])
    # a1 on every partition [128, 16] (128 descriptors), then doubled
    # on-chip to [128, 1024] -- a stride-0 DMA broadcast to 8192 descs
    # costs ~67us of HWDGE sequencer time, so never do that.
    a1rep = const.tile([128, N], F32)
    nc.sync.dma_start(out=a1rep[:, 0:16],
                      in_=bass.AP(tensor=io["a1"].tensor,
                                  offset=io["a1"].offset,
                                  ap=[[0, 128], [1, 16]]))
    for sz in (16, 32, 64, 128, 256, 512):
        nc.gpsimd.tensor_copy(a1rep[:, sz:2 * sz], a1rep[:, 0:sz])
    for k in ("gn2_scale", "gn2_shift", "gn2_alpha"):
        t = const.tile([128, 1], F32, tag=k)
        nc.scalar.dma_start(out=t, in_=io[k])
        gn[k] = t
    Wr2 = const.tile([128, NH2], F32)
    nc.scalar.dma_start(out=Wr2, in_=io["W_r2"])
    # a2 on every partition: [128, 256] (128 x 1KB descriptors, ok)
    a2rep = const.tile([128, NH2], F32)
    nc.scalar.dma_start(out=a2rep,
                        in_=bass.AP(tensor=io["a2"].tensor,
                                    offset=io["a2"].offset,
                                    ap=[[0, 128], [1, NH2]]))

    # ---------------- constants ----------------
    identb = const.tile([128, 128], MMDT)
    make_identity(nc, identb)
    identf = const.tile([128, 128], F32)
    make_identity(nc, identf)
    eps_t = const.tile([128, 1], F32)
    nc.vector.memset(eps_t, EPS)
    neg1 = const.tile([128, 1], F32)
    nc.vector.memset(neg1, -1.0)
    # sel16[p, q] = (p // 8 == q), bf16 -- group-of-8-partitions selector
    sel16 = const.tile([128, 16], MMDT)
    nc.gpsimd.memset(sel16, 0.0)
    nc.gpsimd.affine_select(out=sel16, in_=sel16, compare_op=OP.is_ge,
                            fill=1.0, base=-1, pattern=[[8, 16]],
                            channel_multiplier=-1)
    nc.gpsimd.affine_select(out=sel16, in_=sel16, compare_op=OP.is_ge,
                            fill=0.0, base=7, pattern=[[8, 16]],
                            channel_multiplier=-1)

    # ---------------- layer 1: graph_norm ----------------
    stats = small.tile([128, 2, 6], F32)
    nc.vector.bn_stats(stats[:, 0, :], xg[:, 0:512])
    nc.vector.bn_stats(stats[:, 1, :], xg[:, 512:1024])
    mv = small.tile([128, 2], F32)
    nc.vector.bn_aggr(mv, stats)
    lnv = small.tile([128, 1], F32)
    nc.scalar.activation(lnv, mv[:, 1:2], AF.Ln, bias=eps_t)
    rstd = small.tile([128, 1], F32)
    nc.scalar.activation(rstd, lnv, AF.Exp, scale=-0.5)
    S1 = small.tile([128, 1], F32)
    nc.vector.tensor_mul(S1, rstd, gn["gn1_scale"])
    t0 = small.tile([128, 1], F32)
    nc.vector.tensor_mul(t0, mv[:, 0:1], S1)
    t1 = small.tile([128, 1], F32)
    nc.vector.tensor_mul(t1, t0, gn["gn1_alpha"])
    B1 = small.tile([128, 1], F32)
    nc.vector.tensor_sub(B1, gn["gn1_shift"], t1)
    h1g = big.tile([128, N], F32)
    nc.vector.tensor_scalar(out=h1g, in0=xg, scalar1=S1, scalar2=B1,
                            op0=OP.mult, op1=OP.add)

    # h1T[c, k, p] = h1[8p+k, c]
    h1T = big.tile([128, NT, 128], F32)
    for k in range(NT):
        pst = psA.tile([128, 128], F32, tag="pst")
        nc.tensor.transpose(pst, h1g[:, 128 * k:128 * (k + 1)], identf)
        nc.scalar.copy(h1T[:, k, :], pst)

    # R_all[p, k, :] = leaky(h1 @ W_r1)[8p+k, :]   (fp32 score path)
    R_all = big.tile([128, NT, NH1], F32)
    for k in range(NT):
        psr = psR.tile([128, NH1], F32, tag="psr")
        nc.tensor.matmul(psr, h1T[:, k, :], Wr1, start=True, stop=True)
        rcp = work.tile([128, NH1], F32, tag="rcp1")
        nc.scalar.copy(rcp, psr)
        nc.vector.scalar_tensor_tensor(
            out=R_all[:, k, :], in0=rcp, scalar=SLOPE, in1=rcp,
            op0=OP.mult, op1=OP.max)

    # t[p, (k g)] = sum_d R[p, k, 16g+d] * a1[d];  w = exp(t)
    tmul = big.tile([128, N], F32)
    nc.vector.tensor_mul(tmul, R_all.rearrange("p k c -> p (k c)"), a1rep)
    t_all = big.tile([128, 64], F32)
    nc.vector.tensor_reduce(
        out=t_all, in_=tmul.rearrange("p (q d) -> p q d", d=16),
        axis=mybir.AxisListType.X, op=OP.add)
    w_all = big.tile([128, 64], F32)
    nc.scalar.activation(w_all, t_all, AF.Exp)

    # R17[p, k, 17g+(0:16)] = w*R, R17[p, k, 17g+16] = w   (bf16 values)
    R17 = big.tile([128, NT, 136], MMDT)
    v17 = R17.rearrange("p u (g x) -> p u g x", x=17)
    w3 = w_all.rearrange("p (u g) -> p u g", g=8)
    nc.vector.tensor_mul(v17[:, :, :, 0:16],
                         R_all.rearrange("p u (g d) -> p u g d", d=16),
                         w3.to_broadcast([128, 8, 8, 16]))
    nc.vector.tensor_copy(v17[:, :, :, 16], w3)

    # V1[j'-tile kt] from R17 via DRAM staging (bf16, halves v1's bytes):
    # stage addr A(h,kt,a,b,g,dd) = 17408h + 2176kt + 1088a + 136b + 17g + dd
    vstage = dram.tile([139264], MMDT)
    nc.sync.dma_start(
        out=bass.AP(tensor=vstage.tensor, offset=vstage.offset,
                    ap=[[17408, 8], [1088, 16], [1, 1088]]),
        in_=R17.rearrange("p u c -> p (u c)"))
    V1 = big.tile([128, NT, 136], MMDT)
    for kt in range(NT):
        nc.sync.dma_start(
            out=V1[:, kt, :],
            in_=bass.AP(tensor=vstage.tensor,
                        offset=vstage.offset + 2176 * kt,
                        ap=[[17, 128], [17408, 8], [1, 17]]))

    # ---------------- adj cast + transpose ----------------
    # adjf[k][p, j] = float(adj[8p+k, j]), bf16. int32->f32 on DVE (the
    # one proven convert path), f32->bf16 on ACT.
    adjf = []
    for k in range(NT):
        t = big.tile([128, N], MMDT, tag=f"adjf{k}", name=f"adjf{k}")
        if k % 2 == 0:
            nc.vector.tensor_copy(t, adjpk[:, k, :])
        else:
            nc.gpsimd.tensor_copy(t, adjpk[:, k, :])
        adjf.append(t)

    # adjT[jl, jt, k, p] = adj[8p+k, 128jt+jl]  (lhsT tiles, bf16)
    adjT = big.tile([128, NT, NT, 128], MMDT)
    for k in range(NT):
        for jg in range(0, NT, 4):
            pst = psA.tile([128, 4, 128], MMDT, tag="pst", name=f"pst{k}_{jg}")
            for j in range(4):
                nc.tensor.transpose(pst[:, j, :],
                                    adjf[k][:, 128 * (jg + j):128 * (jg + j + 1)],
                                    identb)
            dst = adjT[:, jg:jg + 4, k, :]
            if (k + jg) % 2 == 0:
                nc.vector.tensor_copy(dst, pst)
            else:
                nc.scalar.copy(dst, pst)


    # ---------------- layer 1: hp = adj @ V1, node-major epilogue ------
    # out tile k rows = nodes i = 8p+k (pseudo 8m+g with g=k), so the
    # normalized/elu'd result IS node-major: out1f[m, h, 16k+d].
    # out1f holds elu+1 (the -1 is folded into B2 / stats downstream).
    out1f = big.tile([128, NT, 128], F32)
    o3 = out1f.rearrange("p h (g d) -> p h g d", d=16)
    for kg in range(0, NT, 4):
        pss = {}
        for k in range(kg, kg + 4):
            pss[k] = psH.tile([128, 136], F32, tag="ps", name=f"hp1_{k}")
        for jt in range(NT):
            for k in range(kg, kg + 4):
                nc.tensor.matmul(pss[k], adjT[:, jt, k, :], V1[:, jt, :],
                                 start=(jt == 0), stop=(jt == NT - 1))
        for k in range(kg, kg + 4):
            ps = pss[k]
            p3 = ps.rearrange("p (h x) -> p h x", x=17)
            rec = work.tile([128, 8], F32, tag="rec1")
            nc.vector.reciprocal(rec, p3[:, :, 16])
            hpn = work.tile([128, 128], F32, tag="hpn")
            nc.vector.tensor_mul(hpn.rearrange("p (h d) -> p h d", d=16),
                                 p3[:, :, 0:16], rec.to_broadcast([128, 8, 16]))
            # elu+1 = relu(x) + exp(min(x, 0)), written strided node-major
            mn = work.tile([128, 128], F32, tag="mn1")
            nc.vector.tensor_scalar_min(out=mn, in0=hpn, scalar1=0.0)
            ex = work.tile([128, 128], F32, tag="ex1")
            nc.scalar.activation(ex, mn, AF.Exp)
            nc.vector.scalar_tensor_tensor(
                out=o3[:, :, k, :],
                in0=hpn.rearrange("p (h d) -> p h d", d=16),
                scalar=0.0, in1=ex.rearrange("p (h d) -> p h d", d=16),
                op0=OP.max, op1=OP.add)

    # bf16 copy + squares (stats / V2-source matmul operands)
    out1b = big.tile([128, N], MMDT)
    nc.vector.tensor_copy(out1b, out1f.rearrange("p h c -> p (h c)"))
    sq = big.tile([128, N], MMDT)
    nc.vector.tensor_mul(sq, out1b, out1b)

    # group sums: psS[q, (h c)] = sum_{p: p//8==q} out1b[p, (h c)]
    s2sum = small.tile([128, 1], F32, tag="s2sum")
    q2sum = small.tile([128, 1], F32, tag="q2sum")
    if _os.environ.get("GAT_STUB_STATS"):
        # crash-bisect stub: fake stats (numerics wrong, mean'=1 var=1)
        nc.vector.memset(s2sum, 1024.0)
        nc.vector.memset(q2sum, 2048.0)
        # keep readers of out1b/sq alive so schedule shape stays similar
        junk = small.tile([128, 1], F32, tag="junk")
        nc.vector.tensor_reduce(out=junk, in_=sq[:, 0:8],
                                axis=mybir.AxisListType.X, op=OP.add)
    else:
        s16 = small.tile([16, 16], F32, tag="s16")
        for srcb, base in ((out1b, 0), (sq, 8)):
            for half in range(2):
                psS = psH.tile([16, 512], F32, tag="ps", name=f"psS{base}_{half}")
                nc.tensor.matmul(psS, sel16, srcb[:, 512 * half:512 * (half + 1)],
                                 start=True, stop=True)
                nc.vector.tensor_reduce(
                    out=s16[:, base + 4 * half:base + 4 * half + 4],
                    in_=psS.rearrange("p (h c) -> p h c", c=128),
                    axis=mybir.AxisListType.X, op=OP.add)
        psst = psA.tile([16, 16], F32, tag="pst")
        nc.tensor.transpose(psst, s16, identf[0:16, 0:16])
        st = small.tile([16, 16], F32, tag="st")
        nc.scalar.copy(st, psst)
        nc.sync.dma_start(out=s2sum, in_=st[0:8, :])
        nc.sync.dma_start(out=q2sum, in_=st[8:16, :])

    # ---------------- layer 2: graph_norm scales ----------------
    # values stored are o1' = elu+1; var is shift-invariant, mean' = mean+1,
    # and B2 absorbs the -1: h2 = S2*o1' + (shift - S2*(1 + alpha*(mean'-1)))
    inv = 1.0 / 1024.0
    mean2 = small.tile([128, 1], F32, tag="mean2")
    nc.vector.tensor_scalar_mul(mean2, s2sum, inv)
    ex2m = small.tile([128, 1], F32, tag="ex2m")
    nc.vector.tensor_scalar_mul(ex2m, q2sum, inv)
    msq = small.tile([128, 1], F32, tag="msq")
    nc.vector.tensor_mul(msq, mean2, mean2)
    var2 = small.tile([128, 1], F32, tag="var2")
    nc.vector.tensor_sub(var2, ex2m, msq)
    lnv2 = small.tile([128, 1], F32, tag="lnv2")
    nc.scalar.activation(lnv2, var2, AF.Ln, bias=eps_t)
    rstd2 = small.tile([128, 1], F32, tag="rstd2")
    nc.scalar.activation(rstd2, lnv2, AF.Exp, scale=-0.5)
    S2 = small.tile([128, 1], F32, tag="S2")
    nc.vector.tensor_mul(S2, rstd2, gn["gn2_scale"])
    m1 = small.tile([128, 1], F32, tag="m1")
    nc.vector.tensor_scalar_add(m1, mean2, -1.0)
    u0 = small.tile([128, 1], F32, tag="u0")
    nc.vector.tensor_mul(u0, m1, gn["gn2_alpha"])
    u1 = small.tile([128, 1], F32, tag="u1")
    nc.vector.tensor_scalar_add(u1, u0, 1.0)
    u2 = small.tile([128, 1], F32, tag="u2")
    nc.vector.tensor_mul(u2, u1, S2)
    B2 = small.tile([128, 1], F32, tag="B2")
    nc.vector.tensor_sub(B2, gn["gn2_shift"], u2)

    # ---------------- layer 2: R2, w2, V2 ----------------
    h2T = big.tile([128, NT, 128], F32)
    R2f = big.tile([128, NT, NH2], F32)
    t2 = big.tile([128, NT], F32)
    sc2 = big.tile([128, NH2], F32)
    for ht in range(NT):
        S2c = work.tile([128, 1], F32, tag="s2c")
        nc.scalar.dma_start(out=S2c,
                            in_=S2[16 * ht:16 * ht + 16, 0].to_broadcast([16, 8]))
        B2c = work.tile([128, 1], F32, tag="b2c")
        nc.scalar.dma_start(out=B2c,
                            in_=B2[16 * ht:16 * ht + 16, 0].to_broadcast([16, 8]))
        h2t = work.tile([128, 128], F32, tag="h2t")
        nc.vector.tensor_scalar(out=h2t, in0=out1f[:, ht, :], scalar1=S2c,
                                scalar2=B2c, op0=OP.mult, op1=OP.add)
        pst = psA.tile([128, 128], F32, tag="pst")
        nc.tensor.transpose(pst, h2t, identf)
        nc.scalar.copy(h2T[:, ht, :], pst)
        psr = psR.tile([128, NH2], F32, tag="psr")
        nc.tensor.matmul(psr, h2T[:, ht, :], Wr2, start=True, stop=True)
        rcp2 = work.tile([128, NH2], F32, tag="rcp2")
        nc.scalar.copy(rcp2, psr)
        nc.vector.scalar_tensor_tensor(
            out=R2f[:, ht, :], in0=rcp2, scalar=SLOPE, in1=rcp2,
            op0=OP.mult, op1=OP.max)
        nc.vector.tensor_mul(sc2, R2f[:, ht, :], a2rep)
        nc.vector.tensor_reduce(out=t2[:, ht:ht + 1], in_=sc2,
                                axis=mybir.AxisListType.X, op=OP.add)
    w2 = big.tile([128, NT], F32)
    nc.scalar.activation(w2, t2, AF.Exp)

    V2 = big.tile([128, NT, NH2 + 1], MMDT)
    for kt in range(NT):
        nc.vector.tensor_scalar_mul(out=V2[:, kt, 0:NH2], in0=R2f[:, kt, :],
                                    scalar1=w2[:, kt:kt + 1])
    nc.vector.tensor_copy(V2[:, :, NH2], w2)

    # ---------------- layer 2: hp2 = adj @ V2, elu, y ----------------
    # out tile k rows = nodes i = 8p+k; y written with strided row DMA.
    yoff = io["y"].offset
    for kg in range(0, NT, 4):
        pss = {}
        for k in range(kg, kg + 4):
            pss[k] = psH.tile([128, NH2 + 1], F32, tag="ps", name=f"hp2_{k}")
        for jt in range(NT):
            for k in range(kg, kg + 4):
                nc.tensor.matmul(pss[k], adjT[:, jt, k, :], V2[:, jt, :],
                                 start=(jt == 0), stop=(jt == NT - 1))
        for k in range(kg, kg + 4):
            ps = pss[k]
            rec2 = work.tile([128, 1], F32, tag="rec2")
            nc.vector.reciprocal(rec2, ps[:, NH2:NH2 + 1])
            y0 = work.tile([128, NH2], F32, tag="y0")
            nc.vector.tensor_scalar_mul(out=y0, in0=ps[:, 0:NH2], scalar1=rec2)
            mn2 = work.tile([128, NH2], F32, tag="mn2")
            nc.vector.tensor_scalar_min(out=mn2, in0=y0, scalar1=0.0)
            ex2 = work.tile([128, NH2], F32, tag="ex2")
            nc.scalar.activation(ex2, mn2, AF.Exp)
            o2 = work.tile([128, NH2], F32, tag="o2")
            nc.vector.scalar_tensor_tensor(
                out=o2, in0=y0, scalar=0.0, in1=ex2, op0=OP.max, op1=OP.add)
            yo = work.tile([128, NH2], F32, tag="yo")
            nc.scalar.activation(yo, o2, AF.Identity, bias=neg1)
            nc.scalar.dma_start(
                out=bass.AP(tensor=io["y"].tensor, offset=yoff + NH2 * k,
                            ap=[[NH2 * 8, 128], [1, NH2]]),
                in_=yo)


def build_program():
    from concourse import bacc

    nc = bacc.Bacc("TRN2", target_bir_lowering=False, debug=False,
                   enable_asserts=True, num_devices=8)
    io = {}
    for name, shape, dt in INPUT_KEYS:
        io[name] = nc.dram_tensor(name, list(shape), dt, kind="ExternalInput").ap()
    io["y"] = nc.dram_tensor("y", [N, NH2], F32, kind="ExternalOutput").ap()
    with tile.TileContext(nc) as tc:
        with ExitStack() as ctx:
            gat_body(ctx, tc, io)
    nc.compile()
    return nc


def _run(inputs, **spmd_kwargs):
    from concourse.bass_utils import run_bass_kernel_spmd

    nc = build_program()
    B = 8
    in_maps = []
    for b in range(B):
        m = {}
        for name, shape, dt in INPUT_KEYS:
            v = np.asarray(inputs[name])
            if name in ("x", "adj"):
                v = v[b]
            m[name] = np.ascontiguousarray(v.reshape(shape),
                                           dtype=mybir.dt.np(dt))
        in_maps.append(m)
    res = run_bass_kernel_spmd(nc, in_maps, core_ids=list(range(B)),
                               **spmd_kwargs)
    out = np.stack([res.results[b]["y"] for b in range(B)], axis=0)
    return out.astype(np.float32), res


def kernel(**inputs) -> np.ndarray:
    return _run(inputs)[0]


# revision 19
# speedup vs baseline: 1.6080x; 1.0223x over previous
"""GATv2 (2-layer, graph-norm) Trainium2 Bass kernel, v2.

B=8 samples of N=1024 nodes; data-parallel one sample per NeuronCore (8
cores). Full inputs in, full output out.

Math notes (same reductions as v1):
- GATv2 score e[i,j] = sl[i] + sr[j]; sl cancels in the row softmax, so
  att[i,:] = adj[i,:]*exp(sr) / (adj[i,:] @ exp(sr)). Left branch never
  needed. exp args are small (|t| < 13), no max subtraction.
- Augmented value matrix V = [w*R | w] makes one adj@V matmul produce
  numerator and denominator together.

v2 layout insight: load adj PACKED [128p, 8k, 1024j] (row 8p+k on
partition p; one DMA, 32KB contiguous descriptors -- the v1 row-major
load had 4KB descriptors and ran at ~60 GB/s aggregate). PE-transpose
slice k of the packed tile directly: out[j, p] = adj[8p+k, j], i.e.
adjT columns i = 8p+k (i-strided). Using those as matmul lhsT makes
output tile k hold rows i = 8p+k = pseudo-nodes 8m+g with g=k -- which
is exactly channel-group k of ALL nodes in node-major order. So layer-1
outputs land node-major with partition = n%128 and NO DRAM staging
round-trip for the output reshuffle (v1 staged 1 MB through DRAM).

Precision: adjacency and V matrices in bf16 (adj 0/1 exact; V errors
~0.4% average out over ~512-neighbor sums); score path (R -> t -> w =
exp(t)) kept fp32 since exp amplifies absolute t errors.
"""
import numpy as np
from contextlib import ExitStack

import concourse.bass as bass
import concourse.tile as tile
import concourse.mybir as mybir
from concourse.masks import make_identity

F32 = mybir.dt.float32
BF16 = mybir.dt.bfloat16
INT32 = mybir.dt.int32
import os as _os
MMDT = F32 if _os.environ.get("GAT_MMDT") == "f32" else BF16
AF = mybir.ActivationFunctionType
OP = mybir.AluOpType

N = 1024
NF = 128
NH1 = 128
NH2 = 256
NT = 8
EPS = 1e-5
SLOPE = 0.2

INPUT_KEYS = [
    ("x", (N, NF), F32),
    ("adj", (N, N), INT32),
    ("W_r1", (NF, NH1), F32),
    ("a1", (16, 1), F32),
    ("W_r2", (NH1, NH2), F32),
    ("a2", (NH2, 1), F32),
    ("gn1_scale", (NF,), F32),
    ("gn1_shift", (NF,), F32),
    ("gn1_alpha", (NF,), F32),
    ("gn2_scale", (NH1,), F32),
    ("gn2_shift", (NH1,), F32),
    ("gn2_alpha", (NH1,), F32),
]


def gat_body(ctx: ExitStack, tc: tile.TileContext, io: dict):
    nc = tc.nc
    const = ctx.enter_context(tc.tile_pool(name="const", bufs=1))
    big = ctx.enter_context(tc.tile_pool(name="big", bufs=1))
    work = ctx.enter_context(tc.tile_pool(name="work", bufs=3))
    small = ctx.enter_context(tc.tile_pool(name="small", bufs=4))
    psA = ctx.enter_context(tc.tile_pool(name="psA", bufs=2, space="PSUM"))
    psR = ctx.enter_context(tc.tile_pool(name="psR", bufs=2, space="PSUM"))
    psH = ctx.enter_context(tc.tile_pool(name="psH", bufs=4, space="PSUM"))
    dram = ctx.enter_context(tc.tile_pool(name="dram", bufs=1, space="DRAM"))

    # ---------------- input DMAs, longest pole first ----------------
    # x flat first (x-branch unblocks early); adj right behind on same ring.
    xg = big.tile([128, N], F32)
    nc.sync.dma_start(out=xg,
                      in_=io["x"].rearrange("(p k) c -> p (k c)", p=128))
    # adj packed: adjpk[p, k, j] = adj[8p+k, j]; one DMA, 32KB descriptors.
    adjpk = big.tile([128, NT, N], INT32)
    nc.sync.dma_start(out=adjpk,
                      in_=io["adj"].rearrange("(p k) j -> p k j", p=128))

    gn = {}
    for k in ("gn1_scale", "gn1_shift", "gn1_alpha"):
        t = const.tile([128, 1], F32, tag=k)
        nc.sync.dma_start(out=t, in_=io[k])
        gn[k] = t
    Wr1 = const.tile([128, NH1], F32)
    nc.sync.dma_start(out=Wr1, in_=io["W_r1"])
    # a1 on every partition [128, 16] (128 descriptors), then doubled
    # on-chip to [128, 1024] -- a stride-0 DMA broadcast to 8192 descs
    # costs ~67us of HWDGE sequencer time, so never do that.
    a1rep = const.tile([128, N], F32)
    nc.sync.dma_start(out=a1rep[:, 0:16],
                      in_=bass.AP(tensor=io["a1"].tensor,
                                  offset=io["a1"].offset,
                                  ap=[[0, 128], [1, 16]]))
    for k in ("gn2_scale", "gn2_shift", "gn2_alpha"):
        t = const.tile([128, 1], F32, tag=k)
        nc.scalar.dma_start(out=t, in_=io[k])
        gn[k] = t
    Wr2 = const.tile([128, NH2], F32)
    nc.scalar.dma_start(out=Wr2, in_=io["W_r2"])
    # a2 on every partition: [128, 256] (128 x 1KB descriptors, ok)
    a2rep = const.tile([128, NH2], F32)
    nc.scalar.dma_start(out=a2rep,
                        in_=bass.AP(tensor=io["a2"].tensor,
                                    offset=io["a2"].offset,
                                    ap=[[0, 128], [1, NH2]]))

    # ---------------- constants ----------------
    identb = const.tile([128, 128], MMDT)
    make_identity(nc, identb)
    identf = const.tile([128, 128], F32)
    make_identity(nc, identf)
    eps_t = const.tile([128, 1], F32)
    nc.vector.memset(eps_t, EPS)
    neg1 = const.tile([128, 1], F32)
    nc.vector.memset(neg1, -1.0)
    # sel16[p, q] = (p // 8 == q), bf16 -- group-of-8-partitions selector
    sel16 = const.tile([128, 16], MMDT)
    nc.gpsimd.memset(sel16, 0.0)
    nc.gpsimd.affine_select(out=sel16, in_=sel16, compare_op=OP.is_ge,
                            fill=1.0, base=-1, pattern=[[8, 16]],
                            channel_multiplier=-1)
    nc.gpsimd.affine_select(out=sel16, in_=sel16, compare_op=OP.is_ge,
                            fill=0.0, base=7, pattern=[[8, 16]],
                            channel_multiplier=-1)
    for sz in (16, 32, 64, 128, 256, 512):
        nc.gpsimd.tensor_copy(a1rep[:, sz:2 * sz], a1rep[:, 0:sz])

    # ---------------- layer 1: graph_norm ----------------
    stats = small.tile([128, 2, 6], F32)
    nc.vector.bn_stats(stats[:, 0, :], xg[:, 0:512])
    nc.vector.bn_stats(stats[:, 1, :], xg[:, 512:1024])
    mv = small.tile([128, 2], F32)
    nc.vector.bn_aggr(mv, stats)
    lnv = small.tile([128, 1], F32)
    nc.scalar.activation(lnv, mv[:, 1:2], AF.Ln, bias=eps_t)
    rstd = small.tile([128, 1], F32)
    nc.scalar.activation(rstd, lnv, AF.Exp, scale=-0.5)
    S1 = small.tile([128, 1], F32)
    nc.vector.tensor_mul(S1, rstd, gn["gn1_scale"])
    t0 = small.tile([128, 1], F32)
    nc.vector.tensor_mul(t0, mv[:, 0:1], S1)
    t1 = small.tile([128, 1], F32)
    nc.vector.tensor_mul(t1, t0, gn["gn1_alpha"])
    B1 = small.tile([128, 1], F32)
    nc.vector.tensor_sub(B1, gn["gn1_shift"], t1)
    h1g = big.tile([128, N], F32)
    nc.vector.tensor_scalar(out=h1g, in0=xg, scalar1=S1, scalar2=B1,
                            op0=OP.mult, op1=OP.add)

    # h1T[c, k, p] = h1[8p+k, c]
    h1T = big.tile([128, NT, 128], F32)
    for k in range(NT):
        pst = psA.tile([128, 128], F32, tag="pst")
        nc.tensor.transpose(pst, h1g[:, 128 * k:128 * (k + 1)], identf)
        nc.scalar.copy(h1T[:, k, :], pst)

    # R_all[p, k, :] = leaky(h1 @ W_r1)[8p+k, :]   (fp32 score path)
    R_all = big.tile([128, NT, NH1], F32)
    for k in range(NT):
        psr = psR.tile([128, NH1], F32, tag="psr")
        nc.tensor.matmul(psr, h1T[:, k, :], Wr1, start=True, stop=True)
        rcp = work.tile([128, NH1], F32, tag="rcp1")
        nc.scalar.copy(rcp, psr)
        nc.vector.scalar_tensor_tensor(
            out=R_all[:, k, :], in0=rcp, scalar=SLOPE, in1=rcp,
            op0=OP.mult, op1=OP.max)

    # t[p, (k g)] = sum_d R[p, k, 16g+d] * a1[d];  w = exp(t)
    tmul = big.tile([128, N], F32)
    nc.vector.tensor_mul(tmul, R_all.rearrange("p k c -> p (k c)"), a1rep)
    t_all = big.tile([128, 64], F32)
    nc.vector.tensor_reduce(
        out=t_all, in_=tmul.rearrange("p (q d) -> p q d", d=16),
        axis=mybir.AxisListType.X, op=OP.add)
    w_all = big.tile([128, 64], F32)
    nc.scalar.activation(w_all, t_all, AF.Exp)

    # R17[p, k, 17g+(0:16)] = w*R, R17[p, k, 17g+16] = w   (bf16 values)
    R17 = big.tile([128, NT, 136], MMDT)
    v17 = R17.rearrange("p u (g x) -> p u g x", x=17)
    w3 = w_all.rearrange("p (u g) -> p u g", g=8)
    nc.vector.tensor_mul(v17[:, :, :, 0:16],
                         R_all.rearrange("p u (g d) -> p u g d", d=16),
                         w3.to_broadcast([128, 8, 8, 16]))
    nc.vector.tensor_copy(v17[:, :, :, 16], w3)

    # V1[j'-tile kt] from R17 via DRAM staging (bf16, halves v1's bytes):
    # stage addr A(h,kt,a,b,g,dd) = 17408h + 2176kt + 1088a + 136b + 17g + dd
    vstage = dram.tile([139264], MMDT)
    nc.sync.dma_start(
        out=bass.AP(tensor=vstage.tensor, offset=vstage.offset,
                    ap=[[17408, 8], [1088, 16], [1, 1088]]),
        in_=R17.rearrange("p u c -> p (u c)"))
    V1 = big.tile([128, NT, 136], MMDT)
    for kt in range(NT):
        nc.sync.dma_start(
            out=V1[:, kt, :],
            in_=bass.AP(tensor=vstage.tensor,
                        offset=vstage.offset + 2176 * kt,
                        ap=[[17, 128], [17408, 8], [1, 17]]))

    # ---------------- adj cast + transpose ----------------
    # adjf[k][p, j] = float(adj[8p+k, j]), bf16. int32->f32 on DVE (the
    # one proven convert path), f32->bf16 on ACT.
    adjf = []
    for k in range(NT):
        t = big.tile([128, N], MMDT, tag=f"adjf{k}", name=f"adjf{k}")
        if k < 6:
            nc.vector.tensor_copy(t, adjpk[:, k, :])
        else:
            nc.gpsimd.tensor_copy(t, adjpk[:, k, :])
        adjf.append(t)

    # adjT[jl, jt, k, p] = adj[8p+k, 128jt+jl]  (lhsT tiles, bf16)
    adjT = big.tile([128, NT, NT, 128], MMDT)
    for k in range(NT):
        for jg in range(0, NT, 4):
            pst = psA.tile([128, 4, 128], MMDT, tag="pst", name=f"pst{k}_{jg}")
            for j in range(4):
                nc.tensor.transpose(pst[:, j, :],
                                    adjf[k][:, 128 * (jg + j):128 * (jg + j + 1)],
                                    identb)
            dst = adjT[:, jg:jg + 4, k, :]
            nc.scalar.copy(dst, pst)


    # ---------------- layer 1: hp = adj @ V1, node-major epilogue ------
    # out tile k rows = nodes i = 8p+k (pseudo 8m+g with g=k), so the
    # normalized/elu'd result IS node-major: out1f[m, h, 16k+d].
    # out1f holds elu+1 (the -1 is folded into B2 / stats downstream).
    out1f = big.tile([128, NT, 128], F32)
    o3 = out1f.rearrange("p h (g d) -> p h g d", d=16)
    for kg in range(0, NT, 4):
        pss = {}
        for k in range(kg, kg + 4):
            pss[k] = psH.tile([128, 136], F32, tag="ps", name=f"hp1_{k}")
        for jt in range(NT):
            for k in range(kg, kg + 4):
                nc.tensor.matmul(pss[k], adjT[:, jt, k, :], V1[:, jt, :],
                                 start=(jt == 0), stop=(jt == NT - 1))
        for k0 in range(kg, kg + 4, 2):
            hpn2 = work.tile([128, 2, 128], F32, tag="hpn")
            for kk in range(2):
                ps = pss[k0 + kk]
                p3 = ps.rearrange("p (h x) -> p h x", x=17)
                rec = work.tile([128, 8], F32, tag="rec1")
                nc.vector.reciprocal(rec, p3[:, :, 16])
                nc.vector.tensor_mul(
                    hpn2[:, kk, :].rearrange("p (h d) -> p h d", d=16),
                    p3[:, :, 0:16], rec.to_broadcast([128, 8, 16]))
            # elu+1 = relu(x) + exp(min(x, 0)), strided node-major write
            mn = work.tile([128, 2, 128], F32, tag="mn1")
            nc.vector.tensor_scalar_min(out=mn, in0=hpn2, scalar1=0.0)
            ex = work.tile([128, 2, 128], F32, tag="ex1")
            nc.scalar.activation(ex, mn, AF.Exp)
            for kk in range(2):
                nc.vector.scalar_tensor_tensor(
                    out=o3[:, :, k0 + kk, :],
                    in0=hpn2[:, kk, :].rearrange("p (h d) -> p h d", d=16),
                    scalar=0.0,
                    in1=ex[:, kk, :].rearrange("p (h d) -> p h d", d=16),
                    op0=OP.max, op1=OP.add)

    # bf16 copy + squares (stats / V2-source matmul operands)
    out1b = big.tile([128, N], MMDT)
    nc.vector.tensor_copy(out1b, out1f.rearrange("p h c -> p (h c)"))
    sq = big.tile([128, N], MMDT)
    nc.vector.tensor_mul(sq, out1b, out1b)

    # group sums: psS[q, (h c)] = sum_{p: p//8==q} out1b[p, (h c)]
    s2sum = small.tile([128, 1], F32, tag="s2sum")
    q2sum = small.tile([128, 1], F32, tag="q2sum")
    if _os.environ.get("GAT_STUB_STATS"):
        # crash-bisect stub: fake stats (numerics wrong, mean'=1 var=1)
        nc.vector.memset(s2sum, 1024.0)
        nc.vector.memset(q2sum, 2048.0)
        # keep readers of out1b/sq alive so schedule shape stays similar
        junk = small.tile([128, 1], F32, tag="junk")
        nc.vector.tensor_reduce(out=junk, in_=sq[:, 0:8],
                                axis=mybir.AxisListType.X, op=OP.add)
    else:
        s16 = small.tile([16, 16], F32, tag="s16")
        for srcb, base in ((out1b, 0), (sq, 8)):
            for half in range(2):
                psS = psH.tile([16, 512], F32, tag="ps", name=f"psS{base}_{half}")
                nc.tensor.matmul(psS, sel16, srcb[:, 512 * half:512 * (half + 1)],
                                 start=True, stop=True)
                nc.vector.tensor_reduce(
                    out=s16[:, base + 4 * half:base + 4 * half + 4],
                    in_=psS.rearrange("p (h c) -> p h c", c=128),
                    axis=mybir.AxisListType.X, op=OP.add)
        psst = psA.tile([16, 16], F32, tag="pst")
        nc.tensor.transpose(psst, s16, identf[0:16, 0:16])
        st = small.tile([16, 16], F32, tag="st")
        nc.scalar.copy(st, psst)
        nc.sync.dma_start(out=s2sum, in_=st[0:8, :])
        nc.sync.dma_start(out=q2sum, in_=st[8:16, :])

    # ---------------- layer 2: graph_norm scales ----------------
    # values stored are o1' = elu+1; var is shift-invariant, mean' = mean+1,
    # and B2 absorbs the -1: h2 = S2*o1' + (shift - S2*(1 + alpha*(mean'-1)))
    inv = 1.0 / 1024.0
    mean2 = small.tile([128, 1], F32, tag="mean2")
    nc.vector.tensor_scalar_mul(mean2, s2sum, inv)
    ex2m = small.tile([128, 1], F32, tag="ex2m")
    nc.vector.tensor_scalar_mul(ex2m, q2sum, inv)
    msq = small.tile([128, 1], F32, tag="msq")
    nc.vector.tensor_mul(msq, mean2, mean2)
    var2 = small.tile([128, 1], F32, tag="var2")
    nc.vector.tensor_sub(var2, ex2m, msq)
    lnv2 = small.tile([128, 1], F32, tag="lnv2")
    nc.scalar.activation(lnv2, var2, AF.Ln, bias=eps_t)
    rstd2 = small.tile([128, 1], F32, tag="rstd2")
    nc.scalar.activation(rstd2, lnv2, AF.Exp, scale=-0.5)
    S2 = small.tile([128, 1], F32, tag="S2")
    nc.vector.tensor_mul(S2, rstd2, gn["gn2_scale"])
    m1 = small.tile([128, 1], F32, tag="m1")
    nc.vector.tensor_scalar_add(m1, mean2, -1.0)
    u0 = small.tile([128, 1], F32, tag="u0")
    nc.vector.tensor_mul(u0, m1, gn["gn2_alpha"])
    u1 = small.tile([128, 1], F32, tag="u1")
    nc.vector.tensor_scalar_add(u1, u0, 1.0)
    u2 = small.tile([128, 1], F32, tag="u2")
    nc.vector.tensor_mul(u2, u1, S2)
    B2 = small.tile([128, 1], F32, tag="B2")
    nc.vector.tensor_sub(B2, gn["gn2_shift"], u2)

    # ---------------- layer 2: R2, w2, V2 ----------------
    h2T = big.tile([128, NT, 128], F32)
    R2f = big.tile([128, NT, NH2], F32)
    t2 = big.tile([128, NT], F32)
    sc2 = big.tile([128, NH2], F32)
    for ht in range(NT):
        S2c = work.tile([128, 1], F32, tag="s2c")
        nc.scalar.dma_start(out=S2c,
                            in_=S2[16 * ht:16 * ht + 16, 0].to_broadcast([16, 8]))
        B2c = work.tile([128, 1], F32, tag="b2c")
        nc.scalar.dma_start(out=B2c,
                            in_=B2[16 * ht:16 * ht + 16, 0].to_broadcast([16, 8]))
        h2t = work.tile([128, 128], F32, tag="h2t")
        nc.scalar.activation(h2t, out1f[:, ht, :], AF.Identity,
                             scale=S2c, bias=B2c)
        pst = psA.tile([128, 128], F32, tag="pst")
        nc.tensor.transpose(pst, h2t, identf)
        nc.scalar.copy(h2T[:, ht, :], pst)
        psr = psR.tile([128, NH2], F32, tag="psr")
        nc.tensor.matmul(psr, h2T[:, ht, :], Wr2, start=True, stop=True)
        rcp2 = work.tile([128, NH2], F32, tag="rcp2")
        nc.scalar.copy(rcp2, psr)
        nc.vector.scalar_tensor_tensor(
            out=R2f[:, ht, :], in0=rcp2, scalar=SLOPE, in1=rcp2,
            op0=OP.mult, op1=OP.max)
        nc.gpsimd.tensor_mul(sc2, R2f[:, ht, :], a2rep)
        nc.vector.tensor_reduce(out=t2[:, ht:ht + 1], in_=sc2,
                                axis=mybir.AxisListType.X, op=OP.add)
    w2 = big.tile([128, NT], F32)
    nc.scalar.activation(w2, t2, AF.Exp)

    V2 = big.tile([128, NT, NH2 + 1], MMDT)
    for kt in range(NT):
        nc.scalar.activation(V2[:, kt, 0:NH2], R2f[:, kt, :], AF.Identity,
                             scale=w2[:, kt:kt + 1])
    nc.vector.tensor_copy(V2[:, :, NH2], w2)

    # ---------------- layer 2: hp2 = adj @ V2, elu, y ----------------
    # out tile k rows = nodes i = 8p+k; y written with strided row DMA.
    yoff = io["y"].offset
    for kg in range(0, NT, 4):
        pss = {}
        for k in range(kg, kg + 4):
            pss[k] = psH.tile([128, NH2 + 1], F32, tag="ps", name=f"hp2_{k}")
        for jt in range(NT):
            for k in range(kg, kg + 4):
                nc.tensor.matmul(pss[k], adjT[:, jt, k, :], V2[:, jt, :],
                                 start=(jt == 0), stop=(jt == NT - 1))
        for k0 in range(kg, kg + 4, 2):
            y02 = work.tile([128, 2, NH2], F32, tag="y0")
            for kk in range(2):
                ps = pss[k0 + kk]
                rec2 = work.tile([128, 1], F32, tag="rec2")
                nc.vector.reciprocal(rec2, ps[:, NH2:NH2 + 1])
                nc.vector.tensor_scalar_mul(out=y02[:, kk, :], in0=ps[:, 0:NH2],
                                            scalar1=rec2)
            mn2 = work.tile([128, 2, NH2], F32, tag="mn2")
            nc.vector.tensor_scalar_min(out=mn2, in0=y02, scalar1=0.0)
            ex2 = work.tile([128, 2, NH2], F32, tag="ex2")
            nc.scalar.activation(ex2, mn2, AF.Exp)
            o2 = work.tile([128, 2, NH2], F32, tag="o2")
            nc.vector.scalar_tensor_tensor(
                out=o2, in0=y02, scalar=0.0, in1=ex2, op0=OP.max, op1=OP.add)
            yo = work.tile([128, 2, NH2], F32, tag="yo")
            nc.scalar.activation(yo, o2, AF.Identity, bias=neg1)
            nc.scalar.dma_start(
                out=bass.AP(tensor=io["y"].tensor, offset=yoff + NH2 * k0,
                            ap=[[NH2 * 8, 128], [NH2, 2], [1, NH2]]),
                in_=yo)


def build_program():
    from concourse import bacc

    nc = bacc.Bacc("TRN2", target_bir_lowering=False, debug=False,
                   enable_asserts=True, num_devices=8)
    io = {}
    for name, shape, dt in INPUT_KEYS:
        io[name] = nc.dram_tensor(name, list(shape), dt, kind="ExternalInput").ap()
    io["y"] = nc.dram_tensor("y", [N, NH2], F32, kind="ExternalOutput").ap()
    with tile.TileContext(nc) as tc:
        with ExitStack() as ctx:
            gat_body(ctx, tc, io)
    nc.compile()
    return nc


def _run(inputs, **spmd_kwargs):
    from concourse.bass_utils import run_bass_kernel_spmd

    nc = build_program()
    B = 8
    in_maps = []
    for b in range(B):
        m = {}
        for name, shape, dt in INPUT_KEYS:
            v = np.asarray(inputs[name])
            if name in ("x", "adj"):
                v = v[b]
            m[name] = np.ascontiguousarray(v.reshape(shape),
                                           dtype=mybir.dt.np(dt))
        in_maps.append(m)
    res = run_bass_kernel_spmd(nc, in_maps, core_ids=list(range(B)),
                               **spmd_kwargs)
    out = np.stack([res.results[b]["y"] for b in range(B)], axis=0)
    return out.astype(np.float32), res


def kernel(**inputs) -> np.ndarray:
    return _run(inputs)[0]


# revision 20
# speedup vs baseline: 1.6954x; 1.0543x over previous
"""GATv2 (2-layer, graph-norm) Trainium2 Bass kernel, v2.

B=8 samples of N=1024 nodes; data-parallel one sample per NeuronCore (8
cores). Full inputs in, full output out.

Math notes (same reductions as v1):
- GATv2 score e[i,j] = sl[i] + sr[j]; sl cancels in the row softmax, so
  att[i,:] = adj[i,:]*exp(sr) / (adj[i,:] @ exp(sr)). Left branch never
  needed. exp args are small (|t| < 13), no max subtraction.
- Augmented value matrix V = [w*R | w] makes one adj@V matmul produce
  numerator and denominator together.

v2 layout insight: load adj PACKED [128p, 8k, 1024j] (row 8p+k on
partition p; one DMA, 32KB contiguous descriptors -- the v1 row-major
load had 4KB descriptors and ran at ~60 GB/s aggregate). PE-transpose
slice k of the packed tile directly: out[j, p] = adj[8p+k, j], i.e.
adjT columns i = 8p+k (i-strided). Using those as matmul lhsT makes
output tile k hold rows i = 8p+k = pseudo-nodes 8m+g with g=k -- which
is exactly channel-group k of ALL nodes in node-major order. So layer-1
outputs land node-major with partition = n%128 and NO DRAM staging
round-trip for the output reshuffle (v1 staged 1 MB through DRAM).

Precision: adjacency and V matrices in bf16 (adj 0/1 exact; V errors
~0.4% average out over ~512-neighbor sums); score path (R -> t -> w =
exp(t)) kept fp32 since exp amplifies absolute t errors.
"""
import numpy as np
from contextlib import ExitStack

import concourse.bass as bass
import concourse.tile as tile
import concourse.mybir as mybir
from concourse.masks import make_identity

F32 = mybir.dt.float32
BF16 = mybir.dt.bfloat16
INT32 = mybir.dt.int32
import os as _os
MMDT = F32 if _os.environ.get("GAT_MMDT") == "f32" else BF16
AF = mybir.ActivationFunctionType
OP = mybir.AluOpType

N = 1024
NF = 128
NH1 = 128
NH2 = 256
NT = 8
EPS = 1e-5
SLOPE = 0.2

INPUT_KEYS = [
    ("x", (N, NF), F32),
    ("adj", (N, N), INT32),
    ("W_r1", (NF, NH1), F32),
    ("a1", (16, 1), F32),
    ("W_r2", (NH1, NH2), F32),
    ("a2", (NH2, 1), F32),
    ("gn1_scale", (NF,), F32),
    ("gn1_shift", (NF,), F32),
    ("gn1_alpha", (NF,), F32),
    ("gn2_scale", (NH1,), F32),
    ("gn2_shift", (NH1,), F32),
    ("gn2_alpha", (NH1,), F32),
]


def gat_body(ctx: ExitStack, tc: tile.TileContext, io: dict):
    nc = tc.nc
    const = ctx.enter_context(tc.tile_pool(name="const", bufs=1))
    big = ctx.enter_context(tc.tile_pool(name="big", bufs=1))
    work = ctx.enter_context(tc.tile_pool(name="work", bufs=3))
    small = ctx.enter_context(tc.tile_pool(name="small", bufs=4))
    psA = ctx.enter_context(tc.tile_pool(name="psA", bufs=2, space="PSUM"))
    psR = ctx.enter_context(tc.tile_pool(name="psR", bufs=2, space="PSUM"))
    psH = ctx.enter_context(tc.tile_pool(name="psH", bufs=4, space="PSUM"))
    dram = ctx.enter_context(tc.tile_pool(name="dram", bufs=1, space="DRAM"))

    # ---------------- input DMAs, longest pole first ----------------
    # x flat first (x-branch unblocks early); adj right behind on same ring.
    xg = big.tile([128, N], F32)
    nc.scalar.dma_start(out=xg,
                        in_=io["x"].rearrange("(p k) c -> p (k c)", p=128))
    # adj packed: adjpk[p, k, j] = adj[8p+k, j]; one DMA, 32KB descriptors.
    adjpk = big.tile([128, NT, N], INT32)
    nc.sync.dma_start(out=adjpk,
                      in_=io["adj"].rearrange("(p k) j -> p k j", p=128))

    gn = {}
    for k in ("gn1_scale", "gn1_shift", "gn1_alpha"):
        t = const.tile([128, 1], F32, tag=k)
        nc.sync.dma_start(out=t, in_=io[k])
        gn[k] = t
    Wr1 = const.tile([128, NH1], F32)
    nc.sync.dma_start(out=Wr1, in_=io["W_r1"])
    # a1 on every partition [128, 16] (128 descriptors), then doubled
    # on-chip to [128, 1024] -- a stride-0 DMA broadcast to 8192 descs
    # costs ~67us of HWDGE sequencer time, so never do that.
    a1rep = const.tile([128, N], F32)
    nc.sync.dma_start(out=a1rep[:, 0:16],
                      in_=bass.AP(tensor=io["a1"].tensor,
                                  offset=io["a1"].offset,
                                  ap=[[0, 128], [1, 16]]))
    for k in ("gn2_scale", "gn2_shift", "gn2_alpha"):
        t = const.tile([128, 1], F32, tag=k)
        nc.scalar.dma_start(out=t, in_=io[k])
        gn[k] = t
    Wr2 = const.tile([128, NH2], F32)
    nc.scalar.dma_start(out=Wr2, in_=io["W_r2"])
    # a2 on every partition: [128, 256] (128 x 1KB descriptors, ok)
    a2rep = const.tile([128, NH2], F32)
    nc.scalar.dma_start(out=a2rep,
                        in_=bass.AP(tensor=io["a2"].tensor,
                                    offset=io["a2"].offset,
                                    ap=[[0, 128], [1, NH2]]))

    # ---------------- constants ----------------
    identb = const.tile([128, 128], MMDT)
    make_identity(nc, identb)
    identf = const.tile([128, 128], F32)
    make_identity(nc, identf)
    eps_t = const.tile([128, 1], F32)
    nc.vector.memset(eps_t, EPS)
    neg1 = const.tile([128, 1], F32)
    nc.vector.memset(neg1, -1.0)
    # sel16[p, q] = (p // 8 == q), bf16 -- group-of-8-partitions selector
    sel16 = const.tile([128, 16], MMDT)
    nc.gpsimd.memset(sel16, 0.0)
    nc.gpsimd.affine_select(out=sel16, in_=sel16, compare_op=OP.is_ge,
                            fill=1.0, base=-1, pattern=[[8, 16]],
                            channel_multiplier=-1)
    nc.gpsimd.affine_select(out=sel16, in_=sel16, compare_op=OP.is_ge,
                            fill=0.0, base=7, pattern=[[8, 16]],
                            channel_multiplier=-1)
    for sz in (16, 32, 64, 128, 256, 512):
        nc.gpsimd.tensor_copy(a1rep[:, sz:2 * sz], a1rep[:, 0:sz])

    # ---------------- layer 1: graph_norm ----------------
    stats = small.tile([128, 2, 6], F32)
    nc.vector.bn_stats(stats[:, 0, :], xg[:, 0:512])
    nc.vector.bn_stats(stats[:, 1, :], xg[:, 512:1024])
    mv = small.tile([128, 2], F32)
    nc.vector.bn_aggr(mv, stats)
    lnv = small.tile([128, 1], F32)
    nc.scalar.activation(lnv, mv[:, 1:2], AF.Ln, bias=eps_t)
    rstd = small.tile([128, 1], F32)
    nc.scalar.activation(rstd, lnv, AF.Exp, scale=-0.5)
    S1 = small.tile([128, 1], F32)
    nc.vector.tensor_mul(S1, rstd, gn["gn1_scale"])
    t0 = small.tile([128, 1], F32)
    nc.vector.tensor_mul(t0, mv[:, 0:1], S1)
    t1 = small.tile([128, 1], F32)
    nc.vector.tensor_mul(t1, t0, gn["gn1_alpha"])
    B1 = small.tile([128, 1], F32)
    nc.vector.tensor_sub(B1, gn["gn1_shift"], t1)
    h1g = big.tile([128, N], F32)
    nc.vector.tensor_scalar(out=h1g, in0=xg, scalar1=S1, scalar2=B1,
                            op0=OP.mult, op1=OP.add)

    # h1T[c, k, p] = h1[8p+k, c]
    h1T = big.tile([128, NT, 128], F32)
    for k in range(NT):
        pst = psA.tile([128, 128], F32, tag="pst")
        nc.tensor.transpose(pst, h1g[:, 128 * k:128 * (k + 1)], identf)
        nc.scalar.copy(h1T[:, k, :], pst)

    # R_all[p, k, :] = leaky(h1 @ W_r1)[8p+k, :]   (fp32 score path)
    R_all = big.tile([128, NT, NH1], F32)
    for k in range(NT):
        psr = psR.tile([128, NH1], F32, tag="psr")
        nc.tensor.matmul(psr, h1T[:, k, :], Wr1, start=True, stop=True)
        rcp = work.tile([128, NH1], F32, tag="rcp1")
        nc.scalar.copy(rcp, psr)
        nc.vector.scalar_tensor_tensor(
            out=R_all[:, k, :], in0=rcp, scalar=SLOPE, in1=rcp,
            op0=OP.mult, op1=OP.max)

    # t[p, (k g)] = sum_d R[p, k, 16g+d] * a1[d];  w = exp(t)
    tmul = big.tile([128, N], F32)
    nc.vector.tensor_mul(tmul, R_all.rearrange("p k c -> p (k c)"), a1rep)
    t_all = big.tile([128, 64], F32)
    nc.vector.tensor_reduce(
        out=t_all, in_=tmul.rearrange("p (q d) -> p q d", d=16),
        axis=mybir.AxisListType.X, op=OP.add)
    w_all = big.tile([128, 64], F32)
    nc.scalar.activation(w_all, t_all, AF.Exp)

    # R17[p, k, 17g+(0:16)] = w*R, R17[p, k, 17g+16] = w   (bf16 values)
    R17 = big.tile([128, NT, 136], MMDT)
    v17 = R17.rearrange("p u (g x) -> p u g x", x=17)
    w3 = w_all.rearrange("p (u g) -> p u g", g=8)
    nc.vector.tensor_mul(v17[:, :, :, 0:16],
                         R_all.rearrange("p u (g d) -> p u g d", d=16),
                         w3.to_broadcast([128, 8, 8, 16]))
    nc.vector.tensor_copy(v17[:, :, :, 16], w3)

    # V1[j'-tile kt] from R17 via DRAM staging (bf16, halves v1's bytes):
    # stage addr A(h,kt,a,b,g,dd) = 17408h + 2176kt + 1088a + 136b + 17g + dd
    vstage = dram.tile([139264], MMDT)
    nc.sync.dma_start(
        out=bass.AP(tensor=vstage.tensor, offset=vstage.offset,
                    ap=[[17408, 8], [1088, 16], [1, 1088]]),
        in_=R17.rearrange("p u c -> p (u c)"))
    V1 = big.tile([128, NT, 136], MMDT)
    for kt in range(NT):
        nc.sync.dma_start(
            out=V1[:, kt, :],
            in_=bass.AP(tensor=vstage.tensor,
                        offset=vstage.offset + 2176 * kt,
                        ap=[[17, 128], [17408, 8], [1, 17]]))

    # ---------------- adj cast + transpose ----------------
    # adjf[k][p, j] = float(adj[8p+k, j]), bf16. int32->f32 on DVE (the
    # one proven convert path), f32->bf16 on ACT.
    adjf = []
    with tc.high_priority(offset=-100000):
        # negative offset = schedule as if issued later: keeps the DVE
        # stream from running these ahead of the score-path ops.
        for k in range(NT):
            t = big.tile([128, N], MMDT, tag=f"adjf{k}", name=f"adjf{k}")
            if k < 6:
                nc.vector.tensor_copy(t, adjpk[:, k, :])
            else:
                nc.gpsimd.tensor_copy(t, adjpk[:, k, :])
            adjf.append(t)

    # adjT[jl, jt, k, p] = adj[8p+k, 128jt+jl]  (lhsT tiles, bf16)
    adjT = big.tile([128, NT, NT, 128], MMDT)
    for k in range(NT):
        for jg in range(0, NT, 4):
            pst = psA.tile([128, 4, 128], MMDT, tag="pst", name=f"pst{k}_{jg}")
            for j in range(4):
                nc.tensor.transpose(pst[:, j, :],
                                    adjf[k][:, 128 * (jg + j):128 * (jg + j + 1)],
                                    identb)
            dst = adjT[:, jg:jg + 4, k, :]
            nc.scalar.copy(dst, pst)


    # ---------------- layer 1: hp = adj @ V1, node-major epilogue ------
    # out tile k rows = nodes i = 8p+k (pseudo 8m+g with g=k), so the
    # normalized/elu'd result IS node-major: out1f[m, h, 16k+d].
    # out1f holds elu+1 (the -1 is folded into B2 / stats downstream).
    out1f = big.tile([128, NT, 128], F32)
    o3 = out1f.rearrange("p h (g d) -> p h g d", d=16)
    for kg in range(0, NT, 4):
        pss = {}
        for k in range(kg, kg + 4):
            pss[k] = psH.tile([128, 136], F32, tag="ps", name=f"hp1_{k}")
        for jt in range(NT):
            for k in range(kg, kg + 4):
                nc.tensor.matmul(pss[k], adjT[:, jt, k, :], V1[:, jt, :],
                                 start=(jt == 0), stop=(jt == NT - 1))
        for k0 in range(kg, kg + 4, 2):
            hpn2 = work.tile([128, 2, 128], F32, tag="hpn")
            for kk in range(2):
                ps = pss[k0 + kk]
                p3 = ps.rearrange("p (h x) -> p h x", x=17)
                rec = work.tile([128, 8], F32, tag="rec1")
                nc.vector.reciprocal(rec, p3[:, :, 16])
                nc.vector.tensor_mul(
                    hpn2[:, kk, :].rearrange("p (h d) -> p h d", d=16),
                    p3[:, :, 0:16], rec.to_broadcast([128, 8, 16]))
            # elu+1 = relu(x) + exp(min(x, 0)), strided node-major write
            mn = work.tile([128, 2, 128], F32, tag="mn1")
            nc.vector.tensor_scalar_min(out=mn, in0=hpn2, scalar1=0.0)
            ex = work.tile([128, 2, 128], F32, tag="ex1")
            nc.scalar.activation(ex, mn, AF.Exp)
            for kk in range(2):
                nc.vector.scalar_tensor_tensor(
                    out=o3[:, :, k0 + kk, :],
                    in0=hpn2[:, kk, :].rearrange("p (h d) -> p h d", d=16),
                    scalar=0.0,
                    in1=ex[:, kk, :].rearrange("p (h d) -> p h d", d=16),
                    op0=OP.max, op1=OP.add)

    # bf16 copy + squares (stats / V2-source matmul operands)
    out1b = big.tile([128, N], MMDT)
    nc.vector.tensor_copy(out1b, out1f.rearrange("p h c -> p (h c)"))
    sq = big.tile([128, N], MMDT)
    nc.vector.tensor_mul(sq, out1b, out1b)

    # group sums: psS[q, (h c)] = sum_{p: p//8==q} out1b[p, (h c)]
    s2sum = small.tile([128, 1], F32, tag="s2sum")
    q2sum = small.tile([128, 1], F32, tag="q2sum")
    if _os.environ.get("GAT_STUB_STATS"):
        # crash-bisect stub: fake stats (numerics wrong, mean'=1 var=1)
        nc.vector.memset(s2sum, 1024.0)
        nc.vector.memset(q2sum, 2048.0)
        # keep readers of out1b/sq alive so schedule shape stays similar
        junk = small.tile([128, 1], F32, tag="junk")
        nc.vector.tensor_reduce(out=junk, in_=sq[:, 0:8],
                                axis=mybir.AxisListType.X, op=OP.add)
    else:
        s16 = small.tile([16, 16], F32, tag="s16")
        for srcb, base in ((out1b, 0), (sq, 8)):
            for half in range(2):
                psS = psH.tile([16, 512], F32, tag="ps", name=f"psS{base}_{half}")
                nc.tensor.matmul(psS, sel16, srcb[:, 512 * half:512 * (half + 1)],
                                 start=True, stop=True)
                nc.vector.tensor_reduce(
                    out=s16[:, base + 4 * half:base + 4 * half + 4],
                    in_=psS.rearrange("p (h c) -> p h c", c=128),
                    axis=mybir.AxisListType.X, op=OP.add)
        psst = psA.tile([16, 16], F32, tag="pst")
        nc.tensor.transpose(psst, s16, identf[0:16, 0:16])
        st = small.tile([16, 16], F32, tag="st")
        nc.scalar.copy(st, psst)
        nc.sync.dma_start(out=s2sum, in_=st[0:8, :])
        nc.sync.dma_start(out=q2sum, in_=st[8:16, :])

    # ---------------- layer 2: graph_norm scales ----------------
    # values stored are o1' = elu+1; var is shift-invariant, mean' = mean+1,
    # and B2 absorbs the -1: h2 = S2*o1' + (shift - S2*(1 + alpha*(mean'-1)))
    inv = 1.0 / 1024.0
    mean2 = small.tile([128, 1], F32, tag="mean2")
    nc.vector.tensor_scalar_mul(mean2, s2sum, inv)
    ex2m = small.tile([128, 1], F32, tag="ex2m")
    nc.vector.tensor_scalar_mul(ex2m, q2sum, inv)
    msq = small.tile([128, 1], F32, tag="msq")
    nc.vector.tensor_mul(msq, mean2, mean2)
    var2 = small.tile([128, 1], F32, tag="var2")
    nc.vector.tensor_sub(var2, ex2m, msq)
    lnv2 = small.tile([128, 1], F32, tag="lnv2")
    nc.scalar.activation(lnv2, var2, AF.Ln, bias=eps_t)
    rstd2 = small.tile([128, 1], F32, tag="rstd2")
    nc.scalar.activation(rstd2, lnv2, AF.Exp, scale=-0.5)
    S2 = small.tile([128, 1], F32, tag="S2")
    nc.vector.tensor_mul(S2, rstd2, gn["gn2_scale"])
    m1 = small.tile([128, 1], F32, tag="m1")
    nc.vector.tensor_scalar_add(m1, mean2, -1.0)
    u0 = small.tile([128, 1], F32, tag="u0")
    nc.vector.tensor_mul(u0, m1, gn["gn2_alpha"])
    u1 = small.tile([128, 1], F32, tag="u1")
    nc.vector.tensor_scalar_add(u1, u0, 1.0)
    u2 = small.tile([128, 1], F32, tag="u2")
    nc.vector.tensor_mul(u2, u1, S2)
    B2 = small.tile([128, 1], F32, tag="B2")
    nc.vector.tensor_sub(B2, gn["gn2_shift"], u2)

    # ---------------- layer 2: R2, w2, V2 ----------------
    h2T = big.tile([128, NT, 128], F32)
    R2f = big.tile([128, NT, NH2], F32)
    t2 = big.tile([128, NT], F32)
    sc2 = big.tile([128, NH2], F32)
    for ht in range(NT):
        S2c = work.tile([128, 1], F32, tag="s2c")
        nc.gpsimd.dma_start(out=S2c,
                            in_=S2[16 * ht:16 * ht + 16, 0].to_broadcast([16, 8]))
        B2c = work.tile([128, 1], F32, tag="b2c")
        nc.gpsimd.dma_start(out=B2c,
                            in_=B2[16 * ht:16 * ht + 16, 0].to_broadcast([16, 8]))
        h2t = work.tile([128, 128], F32, tag="h2t")
        nc.scalar.activation(h2t, out1f[:, ht, :], AF.Identity,
                             scale=S2c, bias=B2c)
        pst = psA.tile([128, 128], F32, tag="pst")
        nc.tensor.transpose(pst, h2t, identf)
        nc.scalar.copy(h2T[:, ht, :], pst)
        psr = psR.tile([128, NH2], F32, tag="psr")
        nc.tensor.matmul(psr, h2T[:, ht, :], Wr2, start=True, stop=True)
        rcp2 = work.tile([128, NH2], F32, tag="rcp2")
        nc.scalar.copy(rcp2, psr)
        nc.vector.scalar_tensor_tensor(
            out=R2f[:, ht, :], in0=rcp2, scalar=SLOPE, in1=rcp2,
            op0=OP.mult, op1=OP.max)
        nc.gpsimd.tensor_mul(sc2, R2f[:, ht, :], a2rep)
        nc.vector.tensor_reduce(out=t2[:, ht:ht + 1], in_=sc2,
                                axis=mybir.AxisListType.X, op=OP.add)
    w2 = big.tile([128, NT], F32)
    nc.scalar.activation(w2, t2, AF.Exp)

    V2 = big.tile([128, NT, NH2 + 1], MMDT)
    for kt in range(NT):
        nc.scalar.activation(V2[:, kt, 0:NH2], R2f[:, kt, :], AF.Identity,
                             scale=w2[:, kt:kt + 1])
    nc.vector.tensor_copy(V2[:, :, NH2], w2)

    # ---------------- layer 2: hp2 = adj @ V2, elu, y ----------------
    # out tile k rows = nodes i = 8p+k; y written with strided row DMA.
    yoff = io["y"].offset
    for kg in range(0, NT, 4):
        pss = {}
        for k in range(kg, kg + 4):
            pss[k] = psH.tile([128, NH2 + 1], F32, tag="ps", name=f"hp2_{k}")
        for jt in range(NT):
            for k in range(kg, kg + 4):
                nc.tensor.matmul(pss[k], adjT[:, jt, k, :], V2[:, jt, :],
                                 start=(jt == 0), stop=(jt == NT - 1))
        for k0 in range(kg, kg + 4, 2):
            y02 = work.tile([128, 2, NH2], F32, tag="y0")
            for kk in range(2):
                ps = pss[k0 + kk]
                rec2 = work.tile([128, 1], F32, tag="rec2")
                nc.vector.reciprocal(rec2, ps[:, NH2:NH2 + 1])
                nc.vector.tensor_scalar_mul(out=y02[:, kk, :], in0=ps[:, 0:NH2],
                                            scalar1=rec2)
            mn2 = work.tile([128, 2, NH2], F32, tag="mn2")
            nc.vector.tensor_scalar_min(out=mn2, in0=y02, scalar1=0.0)
            ex2 = work.tile([128, 2, NH2], F32, tag="ex2")
            nc.scalar.activation(ex2, mn2, AF.Exp)
            o2 = work.tile([128, 2, NH2], F32, tag="o2")
            nc.vector.scalar_tensor_tensor(
                out=o2, in0=y02, scalar=0.0, in1=ex2, op0=OP.max, op1=OP.add)
            yo = work.tile([128, 2, NH2], F32, tag="yo")
            nc.scalar.activation(yo, o2, AF.Identity, bias=neg1)
            nc.scalar.dma_start(
                out=bass.AP(tensor=io["y"].tensor, offset=yoff + NH2 * k0,
                            ap=[[NH2 * 8, 128], [NH2, 2], [1, NH2]]),
                in_=yo)


def build_program():
    from concourse import bacc

    nc = bacc.Bacc("TRN2", target_bir_lowering=False, debug=False,
                   enable_asserts=True, num_devices=8)
    io = {}
    for name, shape, dt in INPUT_KEYS:
        io[name] = nc.dram_tensor(name, list(shape), dt, kind="ExternalInput").ap()
    io["y"] = nc.dram_tensor("y", [N, NH2], F32, kind="ExternalOutput").ap()
    with tile.TileContext(nc) as tc:
        with ExitStack() as ctx:
            gat_body(ctx, tc, io)
    nc.compile()
    return nc


def _run(inputs, **spmd_kwargs):
    from concourse.bass_utils import run_bass_kernel_spmd

    nc = build_program()
    B = 8
    in_maps = []
    for b in range(B):
        m = {}
        for name, shape, dt in INPUT_KEYS:
            v = np.asarray(inputs[name])
            if name in ("x", "adj"):
                v = v[b]
            m[name] = np.ascontiguousarray(v.reshape(shape),
                                           dtype=mybir.dt.np(dt))
        in_maps.append(m)
    res = run_bass_kernel_spmd(nc, in_maps, core_ids=list(range(B)),
                               **spmd_kwargs)
    out = np.stack([res.results[b]["y"] for b in range(B)], axis=0)
    return out.astype(np.float32), res


def kernel(**inputs) -> np.ndarray:
    return _run(inputs)[0]
